# revision 7
# baseline (speedup 1.0000x reference)
"""DistMult edge scoring on 8 Trainium2 NeuronCores — PE one-hot, [e,d] layout.

score[e] = sum_d h[src[e],d] * fwd_rel[etype[e],d] * h[dst[e],d]

Baseline: 3 x 512B-row dma_gathers per edge, descriptor-bound on the 4
SWDGE queues (~11.7 ns/desc/queue -> ~600us/pass). Here only u=h[src] is
gathered (fp16 rows, ~80k descs ~ 234us); v=h[dst] and w=rel[etype] are
built on the otherwise-idle PE:

- Edges are globally sorted by (dst>>7, etype) and dealt round-robin to
  cores, so per-core bucket sizes are identical (ceil(g/8)) -> ONE static
  SPMD program; buckets pad to 128 cols, bin-packed into 4096-col tiles.
- Per 128-edge group (one dst-bucket, lanes on partitions):
    psA[e,d] = Ov(lhsT [j,e]) @ hsb[:,b,:]   (v build, one matmul)
    psB[e,d] = Ow(lhsT [r,e]) @ wsb[:,rc,:]  (w build, +1 accumulating
               matmul when the group straddles a 128-relation chunk)
  One-hots are host-built fp8e4m3 (1.0 exact; fp8 lhsT x fp16 rhs is
  bit-correct on HW), DMA'd per tile on the HWDGE rings (off the SWDGE
  queues). Weight reload per matmul measured free (~104ns/matmul, N=128).
- Per macro of 4 groups (one PSUM bank [128,4,128]):
    sv = psA (ScalarE copy fp32->fp16), m = ut*sv (DVE fp16 2x),
    m *= psB (DVE, one-PSUM-operand); the d-reduction alternates between
    DVE reduce_sum and ScalarE activation-accumulate to balance engines.
- u-gather pad slots use a duplicated index 0 (NOT -1: num_idxs_reg must
  equal the count of non-negative indices and be core-invariant; fp16
  plain gathers are concurrency-safe across queues, transposed ones are
  NOT and were abandoned).

Numerics: fp16 values, fp32 PSUM/reduction. rel err ~4.6e-4.
Measured: ~293us/pass vs ~596-635us baseline (~2x).
"""

import contextlib
import os
import sys

import numpy as np
import ml_dtypes

sys.path.insert(0, "/opt/trn_rl_repo")

import concourse.bass as bass
import concourse.mybir as mybir
from concourse import bacc
from concourse.tile import TileContext

N_NODES = 10000
N_EDGES = 640000
D = 128
NUM_RELS = 500
N_CORES = 8
N_B = 79
N_RC = 4

F32 = mybir.dt.float32
F16 = mybir.dt.float16
F8 = mybir.dt.float8e4
I16 = mybir.dt.int16
FP8 = np.dtype(ml_dtypes.float8_e4m3)

TCOL = 4096
GP_T = TCOL // 128  # 32 groups per tile


def _wrap(ix_tile: np.ndarray) -> np.ndarray:
    t = ix_tile.shape[0]
    a = ix_tile.astype(np.int16).reshape(t // 16, 16).T
    return np.broadcast_to(a[None], (8, 16, t // 16)).reshape(128, t // 16)


def marshal(src, dst, etype):
    """Global sort/deal/pad. Returns (meta, extras, perm).

    meta = (ntiles, xslot, groups) with groups[gid] = (tile, gcol, b, chunks)
    where chunks = [(rc, 'm' or xslot_index), ...];
    perm[c, t*TCOL + col] = global edge id (-1 pad).
    """
    b = (dst >> 7).astype(np.int64)
    order = np.lexsort((etype, b))
    bs = b[order]
    gcounts = np.bincount(bs, minlength=N_B)
    starts = np.concatenate([[0], np.cumsum(gcounts)])
    percore = -(-gcounts // N_CORES)
    nb = 128 * (-(-percore // 128))  # per-core padded bucket cols

    # pack at GROUP granularity: groups fill tiles sequentially; a bucket's
    # groups may span tiles (each group is self-contained)
    gid0 = {}
    ngroups = 0
    for bb in range(N_B):
        if nb[bb] == 0:
            continue
        gid0[bb] = ngroups
        ngroups += nb[bb] // 128
    ntiles = -(-ngroups // GP_T)
    L = ntiles * TCOL

    perm = np.full((N_CORES, L), -1, np.int64)
    for bb in range(N_B):
        g = gcounts[bb]
        if g == 0:
            continue
        s = starts[bb]
        base = gid0[bb] * 128
        p = np.arange(percore[bb])
        for c in range(N_CORES):
            k = p * N_CORES + c
            ok = k < g
            perm[c, base + p[ok]] = order[s + k[ok]]

    # groups + rel-chunk unions (identical across cores by construction)
    rc_all = (etype >> 7).astype(np.int64)
    groups = []
    gid_of_slot = np.full(L, -1, np.int64)
    for bb in range(N_B):
        if nb[bb] == 0:
            continue
        for k in range(nb[bb] // 128):
            gid_flat = gid0[bb] + k
            t, gcol = gid_flat // GP_T, gid_flat % GP_T
            lanes = gid_flat * 128 + np.arange(128)
            rcs = set()
            for c in range(N_CORES):
                pp = perm[c, lanes]
                v = pp >= 0
                if v.any():
                    rcs.update(np.unique(rc_all[pp[v]]).tolist())
            gid = len(groups)
            assert gid == gid_flat, (gid, gid_flat)
            gid_of_slot[lanes] = gid
            # all-pad group (cannot happen for nonzero buckets): dummy chunk
            chunks = sorted(rcs) if rcs else [0]
            groups.append((t, gcol, bb, chunks))

    # assign x-slots per tile for non-primary chunks
    xslot_of = {}  # (gid, rc) -> xslot index (within tile)
    xcount = [0] * ntiles
    for gid, (t, gcol, bb, chunks) in enumerate(groups):
        for rc in chunks[1:]:
            xslot_of[(gid, rc)] = xcount[t]
            xcount[t] += 1
    xslot = max(xcount) if xcount else 0

    gmeta = []
    for gid, (t, gcol, bb, chunks) in enumerate(groups):
        ch = [(chunks[0], -1)] + [(rc, xslot_of[(gid, rc)]) for rc in chunks[1:]]
        gmeta.append((t, gcol, bb, ch))

    # per-core tensors
    extras = []
    prim_rc = np.array([g[3][0][0] for g in gmeta], np.int64)
    for c in range(N_CORES):
        p = perm[c]
        valid = p >= 0
        pc = np.where(valid, p, 0)
        # pad slots use index 0 (not -1): num_idxs_reg must equal the count
        # of non-negative indices, which must be core-invariant
        s_idx = np.where(valid, src[pc], 0).astype(np.int64)
        uix = np.stack(
            [_wrap(s_idx[t * TCOL : (t + 1) * TCOL]) for t in range(ntiles)]
        ).transpose(1, 0, 2).copy()  # [128, ntiles, TCOL//16]
        slots = np.arange(L)
        tsel = slots // TCOL
        csel = slots % TCOL
        gsel = gid_of_slot[slots]
        dj = (dst[pc] & 127).astype(np.int64)
        rl = (etype[pc] & 127).astype(np.int64)
        rcv = rc_all[pc]

        ov = np.zeros((ntiles, 128, TCOL), FP8)
        ov[tsel[valid], dj[valid], csel[valid]] = 1.0

        is_prim = valid & (rcv == prim_rc[np.maximum(gsel, 0)])
        owm = np.zeros((ntiles, 128, TCOL), FP8)
        owm[tsel[is_prim], rl[is_prim], csel[is_prim]] = 1.0

        owx = np.zeros((ntiles, 128, max(xslot, 1) * 128), FP8)
        is_x = valid & ~is_prim
        if is_x.any():
            xs = np.array(
                [xslot_of[(gid_of_slot[s], rcv[s])] for s in slots[is_x]],
                np.int64,
            )
            owx[tsel[is_x], rl[is_x], xs * 128 + (csel[is_x] % 128)] = 1.0
        extras.append({"uix": uix, "ov": ov, "owm": owm, "owx": owx})

    return (ntiles, xslot, gmeta), extras, perm


def marshal_tables(h, fwd_rel):
    h16 = h.astype(np.float16)
    hp = np.zeros((N_B * 128, D), np.float16)
    hp[:N_NODES] = h16
    hsb = hp.reshape(N_B, 128, D).transpose(1, 0, 2).copy()  # [j, b, d]
    wpad = np.zeros((N_RC * 128, D), np.float16)
    wpad[:NUM_RELS] = fwd_rel.astype(np.float16)
    wsb = wpad.reshape(N_RC, 128, D).transpose(1, 0, 2).copy()  # [r, rc, d]
    return h16, hsb, wsb


def build_s3(meta, bufs=4, hw_repeat=0, ut_bufs=3):
    ntiles, xslot, gmeta = meta
    xcols = max(xslot, 1) * 128
    nc = bacc.Bacc(num_swdge_queues=4, dynamic_dma_scratch_size=16384)
    h16 = nc.declare_dram_parameter("h16", [N_NODES, D], F16, isOutput=False)
    hsb = nc.declare_dram_parameter("hsb", [128, N_B, D], F16, isOutput=False)
    wsb = nc.declare_dram_parameter("wsb", [128, N_RC, D], F16, isOutput=False)
    uix = nc.declare_dram_parameter(
        "uix", [128, ntiles, TCOL // 16], I16, isOutput=False)
    ov = nc.declare_dram_parameter("ov", [ntiles, 128, TCOL], F8, isOutput=False)
    owm = nc.declare_dram_parameter("owm", [ntiles, 128, TCOL], F8, isOutput=False)
    owx = nc.declare_dram_parameter("owx", [ntiles, 128, xcols], F8, isOutput=False)
    out = nc.declare_dram_parameter("scores", [ntiles, 128, GP_T], F32, isOutput=True)

    by_tile = {}
    for (t, gcol, bb, ch) in gmeta:
        by_tile.setdefault(t, {})[gcol] = (bb, ch)

    with TileContext(nc) as tc:
        with (
            tc.tile_pool(name="wts", bufs=1) as wp,
            tc.tile_pool(name="io", bufs=bufs) as iop,
            tc.tile_pool(name="ug", bufs=ut_bufs) as ugp,
            tc.tile_pool(name="sv", bufs=6) as svp,
            tc.tile_pool(name="m", bufs=6) as mp,
            tc.tile_pool(name="stg", bufs=3) as sp,
            tc.tile_pool(name="ps", bufs=4, space=bass.MemorySpace.PSUM) as pab,
        ):
            hsbt = wp.tile([128, N_B, D], F16, tag="hsb")
            wsbt = wp.tile([128, N_RC, D], F16, tag="wsb")
            # all index tiles preloaded once: SWDGE desc-gen never waits on
            # a per-tile ix DMA, so the 4 queue contexts run continuously
            ixall = wp.tile([128, ntiles, TCOL // 16], I16, tag="ixall")
            nc.sync.dma_start(out=hsbt[:], in_=hsb[:])
            nc.sync.dma_start(out=wsbt[:], in_=wsb[:])
            nc.sync.dma_start(out=ixall[:], in_=uix[:])

            loop_ctx = (
                tc.For_i(0, hw_repeat) if hw_repeat else contextlib.nullcontext()
            )
            with loop_ctx:
                q = 0
                for t in range(ntiles):
                    ut = ugp.tile([128, GP_T, D], F16, tag="ut")
                    for k in range(2):
                        nc.gpsimd.dma_gather(
                            out_ap=ut[:, k * 16 : (k + 1) * 16, :],
                            in_ap=h16[:],
                            idxs_ap=ixall[:, t, k * 128 : (k + 1) * 128],
                            num_idxs=2048, num_idxs_reg=2048, elem_size=D,
                            single_packet=False, queue_num=q % 4,
                        )
                        q += 1
                    ovt = iop.tile([128, TCOL], F8, tag="ovt")
                    owmt = iop.tile([128, TCOL], F8, tag="owmt")
                    owxt = iop.tile([128, xcols], F8, tag="owxt")
                    nc.sync.dma_start(out=ovt[:], in_=ov[t])
                    nc.sync.dma_start(out=owmt[:], in_=owm[t])
                    nc.sync.dma_start(out=owxt[:], in_=owx[t])

                    stag = sp.tile([128, GP_T], F32, tag="stg")
                    tgroups = by_tile.get(t, {})
                    n_g = len(tgroups)
                    # present groups are a contiguous prefix 0..n_g-1
                    assert sorted(tgroups) == list(range(n_g)), (t, sorted(tgroups))
                    for mq in range(GP_T // 4):
                        npres = min(4, n_g - mq * 4)
                        if npres <= 0:
                            break
                        psA = pab.tile([128, 4, D], F32, tag="psA")
                        psB = pab.tile([128, 4, D], F32, tag="psB")
                        for i in range(npres):
                            g = mq * 4 + i
                            bb, ch = tgroups[g]
                            gsl = slice(g * 128, (g + 1) * 128)
                            nc.tensor.matmul(
                                psA[:, i, :], ovt[:, gsl], hsbt[:, bb, :])
                            for ci, (rc, xs) in enumerate(ch):
                                lhs = (
                                    owmt[:, gsl] if xs < 0
                                    else owxt[:, xs * 128 : (xs + 1) * 128]
                                )
                                nc.tensor.matmul(
                                    psB[:, i, :], lhs, wsbt[:, rc, :],
                                    start=(ci == 0), stop=(ci == len(ch) - 1),
                                )
                        sv = svp.tile([128, 4, D], F16, tag="sv")
                        nc.scalar.activation(
                            out=sv[:, :npres, :], in_=psA[:, :npres, :],
                            func=mybir.ActivationFunctionType.Copy,
                        )
                        mm = mp.tile([128, 4, D], F16, tag="mm")
                        nc.vector.tensor_mul(
                            mm[:, :npres, :],
                            ut[:, mq * 4 : mq * 4 + npres, :],
                            sv[:, :npres, :])
                        nc.vector.tensor_mul(
                            mm[:, :npres, :], mm[:, :npres, :],
                            psB[:, :npres, :])
                        if mq % 2 == 0:
                            nc.vector.reduce_sum(
                                stag[:, mq * 4 : mq * 4 + npres],
                                mm[:, :npres, :],
                                axis=mybir.AxisListType.X,
                            )
                        else:
                            # balance: odd macros reduce on ScalarE (per
                            # group, activation free-dim accumulate)
                            dump = svp.tile([128, 4, D], F16, tag="dump")
                            for i in range(npres):
                                nc.scalar.activation(
                                    out=dump[:, i, :], in_=mm[:, i, :],
                                    func=mybir.ActivationFunctionType.Copy,
                                    accum_out=stag[
                                        :, mq * 4 + i : mq * 4 + i + 1],
                                )
                    if n_g < GP_T:
                        # fill dropped tail cols so the DMA-out reads
                        # initialized SBUF (on DVE: Pool engine is dedicated
                        # to SWDGE desc-gen)
                        nc.vector.memset(stag[:, n_g:], 0.0)
                    nc.sync.dma_start(out=out[t], in_=stag[:])

    nc.compile()
    return nc


_CACHE = {}
LAST_RESULTS = None


def kernel(h, src, dst, etype, fwd_rel, rev_rel=None):
    global LAST_RESULTS
    from concourse.bass_utils import run_bass_kernel_spmd

    h = np.asarray(h, dtype=np.float32)
    fwd_rel = np.asarray(fwd_rel, dtype=np.float32)
    src = np.asarray(src).astype(np.int64)
    dst = np.asarray(dst).astype(np.int64)
    etype = np.asarray(etype).astype(np.int64)

    meta, extras, perm = marshal(src, dst, etype)
    key = (meta[0], meta[1], tuple(
        (t, g, b, tuple(map(tuple, ch))) for (t, g, b, ch) in meta[2]))
    if key not in _CACHE:
        _CACHE[key] = build_s3(meta)
    nc = _CACHE[key]

    h16, hsb, wsb = marshal_tables(h, fwd_rel)
    in_maps = [
        {"h16": h16, "hsb": hsb, "wsb": wsb, **extras[c]} for c in range(N_CORES)
    ]
    res = run_bass_kernel_spmd(
        nc, in_maps, core_ids=list(range(N_CORES)),
        trace=bool(os.environ.get("KERNEL_TRACE")),
    )
    LAST_RESULTS = res

    ntiles = meta[0]
    scores = np.empty(N_EDGES, np.float32)
    for c in range(N_CORES):
        got = res.results[c]["scores"]  # [nt, 128(lane), 32(g)]
        flat = got.transpose(0, 2, 1).reshape(-1)  # slot = t*4096 + g*128 + lane
        p = perm[c]
        v = p >= 0
        scores[p[v]] = flat[v]
    return scores



# revision 16
# speedup vs baseline: 1.6901x; 1.6901x over previous
"""DistMult edge scoring on 8 Trainium2 NeuronCores — PE one-hot, [e,d] layout.

score[e] = sum_d h[src[e],d] * fwd_rel[etype[e],d] * h[dst[e],d]

Baseline: 3 x 512B-row dma_gathers per edge, descriptor-bound on the 4
SWDGE queues (~11.7 ns/desc/queue -> ~600us/pass). Here only u=h[src] is
gathered (fp16 rows, ~80k descs ~ 234us); v=h[dst] and w=rel[etype] are
built on the otherwise-idle PE:

- Edges are globally sorted by (dst>>7, etype) and dealt round-robin to
  cores, so per-core bucket sizes are identical (ceil(g/8)) -> ONE static
  SPMD program; buckets pad to 128 cols, bin-packed into 4096-col tiles.
- Per 128-edge group (one dst-bucket, lanes on partitions):
    psA[e,d] = Ov(lhsT [j,e]) @ hsb[:,b,:]   (v build, one matmul)
    psB[e,d] = Ow(lhsT [r,e]) @ wsb[:,rc,:]  (w build, +1 accumulating
               matmul when the group straddles a 128-relation chunk)
  One-hots are host-built fp8e4m3 (1.0 exact; fp8 lhsT x fp16 rhs is
  bit-correct on HW), DMA'd per tile on the HWDGE rings (off the SWDGE
  queues). Weight reload per matmul measured free (~104ns/matmul, N=128).
- Per macro of 4 groups (one PSUM bank [128,4,128]):
    sv = psA (ScalarE copy fp32->fp16), m = ut*sv (DVE fp16 2x),
    m *= psB (DVE, one-PSUM-operand); the d-reduction alternates between
    DVE reduce_sum and ScalarE activation-accumulate to balance engines.
- u-gather pad slots use a duplicated index 0 (NOT -1: num_idxs_reg must
  equal the count of non-negative indices and be core-invariant; fp16
  plain gathers are concurrency-safe across queues, transposed ones are
  NOT and were abandoned).

Numerics: fp16 values, fp32 PSUM/reduction. rel err ~4.6e-4.
Measured: ~293us/pass vs ~596-635us baseline (~2x).
"""

import contextlib
import os
import sys

import numpy as np
import ml_dtypes

sys.path.insert(0, "/opt/trn_rl_repo")

import concourse.bass as bass
import concourse.mybir as mybir
from concourse import bacc
from concourse.tile import TileContext

N_NODES = 10000
N_EDGES = 640000
D = 128
NUM_RELS = 500
N_CORES = 8
N_B = 79
N_RC = 4

F32 = mybir.dt.float32
F16 = mybir.dt.float16
F8 = mybir.dt.float8e4
I16 = mybir.dt.int16
FP8 = np.dtype(ml_dtypes.float8_e4m3)

TCOL = 4096
GP_T = TCOL // 128  # 32 groups per tile


def _wrap(ix_tile: np.ndarray) -> np.ndarray:
    t = ix_tile.shape[0]
    a = ix_tile.astype(np.int16).reshape(t // 16, 16).T
    return np.broadcast_to(a[None], (8, 16, t // 16)).reshape(128, t // 16)


def marshal(src, dst, etype):
    """Global sort/deal/bin-pack. Returns (meta, extras, perm).

    meta = (ntiles, xslot, vxslot, groups) with
    groups[gid] = (tile, gcol, segs, chunks):
      segs   = [(bucket, vxslot_or_-1), ...]  (psA matmuls; buckets are
               bin-packed so a 128-slot group may straddle a boundary)
      chunks = [(rc, xslot_or_-1), ...]       (psB matmuls)
    perm[c, t*TCOL + col] = global edge id (-1 pad).
    """
    b = (dst >> 7).astype(np.int64)
    order = np.lexsort((etype, b))
    bs = b[order]
    gcounts = np.bincount(bs, minlength=N_B)
    starts = np.concatenate([[0], np.cumsum(gcounts)])
    percore = -(-gcounts // N_CORES)
    # bin-packed per-core slot layout: bucket b at [sb[b], sb[b]+percore[b]),
    # no 128-alignment (straddle groups get extra accumulating psA matmuls)
    sb = np.concatenate([[0], np.cumsum(percore)])
    l_used = int(sb[-1])
    ngroups = -(-l_used // 128)
    ntiles = -(-ngroups // GP_T)
    L = ntiles * TCOL

    perm = np.full((N_CORES, L), -1, np.int64)
    bucket_of_slot = np.full(L, -1, np.int64)
    for bb in range(N_B):
        g = gcounts[bb]
        if g == 0:
            continue
        s = starts[bb]
        base = int(sb[bb])
        bucket_of_slot[base : base + percore[bb]] = bb
        p = np.arange(percore[bb])
        for c in range(N_CORES):
            k = p * N_CORES + c
            ok = k < g
            perm[c, base + p[ok]] = order[s + k[ok]]

    # groups: bucket segments + rel-chunk unions (core-invariant)
    rc_all = (etype >> 7).astype(np.int64)
    groups = []
    for gid in range(ngroups):
        t, gcol = gid // GP_T, gid % GP_T
        lanes = gid * 128 + np.arange(128)
        bos = bucket_of_slot[lanes]
        segs = []
        for bb in np.unique(bos):
            if bb >= 0:
                segs.append(int(bb))
        if not segs:
            segs = [0]  # all-pad group: dummy segment
        rcs = set()
        for c in range(N_CORES):
            pp = perm[c, lanes]
            v = pp >= 0
            if v.any():
                rcs.update(np.unique(rc_all[pp[v]]).tolist())
        chunks = sorted(rcs) if rcs else [0]
        groups.append((t, gcol, segs, chunks))

    # x-slots per tile: psB non-primary chunks (owx) and psA non-primary
    # bucket segments (ovx) get side one-hot blocks
    xslot_of, vxslot_of = {}, {}
    xcount = [0] * ntiles
    vxcount = [0] * ntiles
    for gid, (t, gcol, segs, chunks) in enumerate(groups):
        for rc in chunks[1:]:
            xslot_of[(gid, rc)] = xcount[t]
            xcount[t] += 1
        for bb in segs[1:]:
            vxslot_of[(gid, bb)] = vxcount[t]
            vxcount[t] += 1
    xslot = max(xcount) if xcount else 0
    vxslot = max(vxcount) if vxcount else 0

    gmeta = []
    for gid, (t, gcol, segs, chunks) in enumerate(groups):
        sg = [(segs[0], -1)] + [(bb, vxslot_of[(gid, bb)]) for bb in segs[1:]]
        ch = [(chunks[0], -1)] + [(rc, xslot_of[(gid, rc)]) for rc in chunks[1:]]
        gmeta.append((t, gcol, sg, ch))

    # per-core tensors
    extras = []
    prim_rc = np.array([g[3][0][0] for g in gmeta], np.int64)
    prim_bb = np.array([g[2][0][0] for g in gmeta], np.int64)
    for c in range(N_CORES):
        p = perm[c]
        valid = p >= 0
        pc = np.where(valid, p, 0)
        # pad slots use index 0 (not -1): num_idxs_reg must equal the count
        # of non-negative indices, which must be core-invariant
        s_idx = np.where(valid, src[pc], 0).astype(np.int64)
        uix = np.stack(
            [_wrap(s_idx[t * TCOL : (t + 1) * TCOL]) for t in range(ntiles)]
        )
        slots = np.arange(L)
        tsel = slots // TCOL
        csel = slots % TCOL
        # tail slots past the last real group are pads (valid=False); clip
        # so the prim_* lookups stay in bounds
        gsel = np.minimum(slots // 128, len(gmeta) - 1)
        dj = (dst[pc] & 127).astype(np.int64)
        rl = (etype[pc] & 127).astype(np.int64)
        rcv = rc_all[pc]
        bv = bucket_of_slot[slots]

        is_pb = valid & (bv == prim_bb[gsel])
        ov = np.zeros((ntiles, 128, TCOL), FP8)
        ov[tsel[is_pb], dj[is_pb], csel[is_pb]] = 1.0

        ovx = np.zeros((ntiles, 128, max(vxslot, 1) * 128), FP8)
        is_vx = valid & ~is_pb
        if is_vx.any():
            vxs = np.array(
                [vxslot_of[(gsel[s], bv[s])] for s in slots[is_vx]],
                np.int64,
            )
            ovx[tsel[is_vx], dj[is_vx], vxs * 128 + (csel[is_vx] % 128)] = 1.0

        is_prim = valid & (rcv == prim_rc[gsel])
        owm = np.zeros((ntiles, 128, TCOL), FP8)
        owm[tsel[is_prim], rl[is_prim], csel[is_prim]] = 1.0

        owx = np.zeros((ntiles, 128, max(xslot, 1) * 128), FP8)
        is_x = valid & ~is_prim
        if is_x.any():
            xs = np.array(
                [xslot_of[(gsel[s], rcv[s])] for s in slots[is_x]],
                np.int64,
            )
            owx[tsel[is_x], rl[is_x], xs * 128 + (csel[is_x] % 128)] = 1.0
        extras.append(
            {"uix": uix, "ov": ov, "ovx": ovx, "owm": owm, "owx": owx})

    return (ntiles, xslot, vxslot, gmeta), extras, perm


def marshal_tables(h, fwd_rel):
    h16 = h.astype(np.float16)
    hp = np.zeros((N_B * 128, D), np.float16)
    hp[:N_NODES] = h16
    hsb = hp.reshape(N_B, 128, D).transpose(1, 0, 2).copy()  # [j, b, d]
    wpad = np.zeros((N_RC * 128, D), np.float16)
    wpad[:NUM_RELS] = fwd_rel.astype(np.float16)
    wsb = wpad.reshape(N_RC, 128, D).transpose(1, 0, 2).copy()  # [r, rc, d]
    return h16, hsb, wsb


def build_s3(meta, bufs=4, hw_repeat=0):
    ntiles, xslot, vxslot, gmeta = meta
    xcols = max(xslot, 1) * 128
    vxcols = max(vxslot, 1) * 128
    nc = bacc.Bacc(num_swdge_queues=4, dynamic_dma_scratch_size=16384)
    h16 = nc.declare_dram_parameter("h16", [N_NODES, D], F16, isOutput=False)
    hsb = nc.declare_dram_parameter("hsb", [128, N_B, D], F16, isOutput=False)
    wsb = nc.declare_dram_parameter("wsb", [128, N_RC, D], F16, isOutput=False)
    uix = nc.declare_dram_parameter(
        "uix", [ntiles, 128, TCOL // 16], I16, isOutput=False)
    ov = nc.declare_dram_parameter("ov", [ntiles, 128, TCOL], F8, isOutput=False)
    ovx = nc.declare_dram_parameter("ovx", [ntiles, 128, vxcols], F8, isOutput=False)
    owm = nc.declare_dram_parameter("owm", [ntiles, 128, TCOL], F8, isOutput=False)
    owx = nc.declare_dram_parameter("owx", [ntiles, 128, xcols], F8, isOutput=False)
    out = nc.declare_dram_parameter("scores", [ntiles, 128, GP_T], F32, isOutput=True)

    by_tile = {}
    for (t, gcol, sg, ch) in gmeta:
        by_tile.setdefault(t, {})[gcol] = (sg, ch)

    with TileContext(nc) as tc:
        with (
            tc.tile_pool(name="wts", bufs=1) as wp,
            tc.tile_pool(name="io", bufs=bufs) as iop,
            tc.tile_pool(name="ixp", bufs=8) as ixp,
            tc.tile_pool(name="sv", bufs=6) as svp,
            tc.tile_pool(name="m", bufs=6) as mp,
            tc.tile_pool(name="stg", bufs=3) as sp,
            tc.tile_pool(name="ps", bufs=4, space=bass.MemorySpace.PSUM) as pab,
        ):
            hsbt = wp.tile([128, N_B, D], F16, tag="hsb")
            wsbt = wp.tile([128, N_RC, D], F16, tag="wsb")
            nc.sync.dma_start(out=hsbt[:], in_=hsb[:])
            nc.sync.dma_start(out=wsbt[:], in_=wsb[:])

            loop_ctx = (
                tc.For_i(0, hw_repeat) if hw_repeat else contextlib.nullcontext()
            )
            with loop_ctx:
                q = 0
                for t in range(ntiles):
                    # ix in its own deep pool: the 64KB index DMA runs many
                    # tiles ahead, so SWDGE desc-gen never waits on it; ut
                    # stays in iop (bufs=4) to keep gather in-flight bounded
                    ix = ixp.tile([128, TCOL // 16], I16, tag="ix")
                    nc.sync.dma_start(out=ix[:], in_=uix[t])
                    ut = iop.tile([128, GP_T, D], F16, tag="ut")
                    for k in range(2):
                        nc.gpsimd.dma_gather(
                            out_ap=ut[:, k * 16 : (k + 1) * 16, :],
                            in_ap=h16[:],
                            idxs_ap=ix[:, k * 128 : (k + 1) * 128],
                            num_idxs=2048, num_idxs_reg=2048, elem_size=D,
                            single_packet=False, queue_num=q % 4,
                        )
                        q += 1
                    ovt = iop.tile([128, TCOL], F8, tag="ovt")
                    ovxt = iop.tile([128, vxcols], F8, tag="ovxt")
                    owmt = iop.tile([128, TCOL], F8, tag="owmt")
                    owxt = iop.tile([128, xcols], F8, tag="owxt")
                    nc.sync.dma_start(out=ovt[:], in_=ov[t])
                    nc.sync.dma_start(out=ovxt[:], in_=ovx[t])
                    nc.sync.dma_start(out=owmt[:], in_=owm[t])
                    nc.sync.dma_start(out=owxt[:], in_=owx[t])

                    stag = sp.tile([128, GP_T], F32, tag="stg")
                    tgroups = by_tile.get(t, {})
                    n_g = len(tgroups)
                    # present groups are a contiguous prefix 0..n_g-1
                    assert sorted(tgroups) == list(range(n_g)), (t, sorted(tgroups))
                    for mq in range(GP_T // 4):
                        npres = min(4, n_g - mq * 4)
                        if npres <= 0:
                            break
                        psA = pab.tile([128, 4, D], F32, tag="psA")
                        psB = pab.tile([128, 4, D], F32, tag="psB")
                        for i in range(npres):
                            g = mq * 4 + i
                            sg, ch = tgroups[g]
                            gsl = slice(g * 128, (g + 1) * 128)
                            for si, (bb, vxs) in enumerate(sg):
                                lhsa = (
                                    ovt[:, gsl] if vxs < 0
                                    else ovxt[:, vxs * 128 : (vxs + 1) * 128]
                                )
                                nc.tensor.matmul(
                                    psA[:, i, :], lhsa, hsbt[:, bb, :],
                                    start=(si == 0), stop=(si == len(sg) - 1),
                                )
                            for ci, (rc, xs) in enumerate(ch):
                                lhs = (
                                    owmt[:, gsl] if xs < 0
                                    else owxt[:, xs * 128 : (xs + 1) * 128]
                                )
                                nc.tensor.matmul(
                                    psB[:, i, :], lhs, wsbt[:, rc, :],
                                    start=(ci == 0), stop=(ci == len(ch) - 1),
                                )
                        sv = svp.tile([128, 4, D], F16, tag="sv")
                        nc.scalar.activation(
                            out=sv[:, :npres, :], in_=psA[:, :npres, :],
                            func=mybir.ActivationFunctionType.Copy,
                        )
                        mm = mp.tile([128, 4, D], F16, tag="mm")
                        nc.vector.tensor_mul(
                            mm[:, :npres, :],
                            ut[:, mq * 4 : mq * 4 + npres, :],
                            sv[:, :npres, :])
                        nc.vector.tensor_mul(
                            mm[:, :npres, :], mm[:, :npres, :],
                            psB[:, :npres, :])
                        if mq % 2 == 0:
                            nc.vector.reduce_sum(
                                stag[:, mq * 4 : mq * 4 + npres],
                                mm[:, :npres, :],
                                axis=mybir.AxisListType.X,
                            )
                        else:
                            # balance: odd macros reduce on ScalarE (per
                            # group, activation free-dim accumulate)
                            dump = svp.tile([128, 4, D], F16, tag="dump")
                            for i in range(npres):
                                nc.scalar.activation(
                                    out=dump[:, i, :], in_=mm[:, i, :],
                                    func=mybir.ActivationFunctionType.Copy,
                                    accum_out=stag[
                                        :, mq * 4 + i : mq * 4 + i + 1],
                                )
                    if n_g < GP_T:
                        # fill dropped tail cols so the DMA-out reads
                        # initialized SBUF
                        nc.gpsimd.memset(stag[:, n_g:], 0.0)
                    nc.sync.dma_start(out=out[t], in_=stag[:])

    nc.compile()
    return nc


_CACHE = {}
LAST_RESULTS = None


def kernel(h, src, dst, etype, fwd_rel, rev_rel=None):
    global LAST_RESULTS
    from concourse.bass_utils import run_bass_kernel_spmd

    h = np.asarray(h, dtype=np.float32)
    fwd_rel = np.asarray(fwd_rel, dtype=np.float32)
    src = np.asarray(src).astype(np.int64)
    dst = np.asarray(dst).astype(np.int64)
    etype = np.asarray(etype).astype(np.int64)

    meta, extras, perm = marshal(src, dst, etype)
    key = (meta[0], meta[1], meta[2], tuple(
        (t, g, tuple(map(tuple, sg)), tuple(map(tuple, ch)))
        for (t, g, sg, ch) in meta[3]))
    if key not in _CACHE:
        _CACHE[key] = build_s3(meta)
    nc = _CACHE[key]

    h16, hsb, wsb = marshal_tables(h, fwd_rel)
    in_maps = [
        {"h16": h16, "hsb": hsb, "wsb": wsb, **extras[c]} for c in range(N_CORES)
    ]
    res = run_bass_kernel_spmd(
        nc, in_maps, core_ids=list(range(N_CORES)),
        trace=bool(os.environ.get("KERNEL_TRACE")),
    )
    LAST_RESULTS = res

    ntiles = meta[0]
    scores = np.empty(N_EDGES, np.float32)
    for c in range(N_CORES):
        got = res.results[c]["scores"]  # [nt, 128(lane), 32(g)]
        flat = got.transpose(0, 2, 1).reshape(-1)  # slot = t*4096 + g*128 + lane
        p = perm[c]
        v = p >= 0
        scores[p[v]] = flat[v]
    return scores



# revision 18
# speedup vs baseline: 1.7082x; 1.0107x over previous
"""DistMult edge scoring on 8 Trainium2 NeuronCores — PE one-hot, [e,d] layout.

score[e] = sum_d h[src[e],d] * fwd_rel[etype[e],d] * h[dst[e],d]

Baseline: 3 x 512B-row dma_gathers per edge, descriptor-bound on the 4
SWDGE queues (~11.7 ns/desc/queue -> ~600us/pass). Here only u=h[src] is
gathered (fp16 rows, ~80k descs ~ 234us); v=h[dst] and w=rel[etype] are
built on the otherwise-idle PE:

- Edges are globally sorted by (dst>>7, etype) and dealt round-robin to
  cores, so per-core bucket sizes are identical (ceil(g/8)) -> ONE static
  SPMD program; buckets pad to 128 cols, bin-packed into 4096-col tiles.
- Per 128-edge group (one dst-bucket, lanes on partitions):
    psA[e,d] = Ov(lhsT [j,e]) @ hsb[:,b,:]   (v build, one matmul)
    psB[e,d] = Ow(lhsT [r,e]) @ wsb[:,rc,:]  (w build, +1 accumulating
               matmul when the group straddles a 128-relation chunk)
  One-hots are host-built fp8e4m3 (1.0 exact; fp8 lhsT x fp16 rhs is
  bit-correct on HW), DMA'd per tile on the HWDGE rings (off the SWDGE
  queues). Weight reload per matmul measured free (~104ns/matmul, N=128).
- Per macro of 4 groups (one PSUM bank [128,4,128]):
    sv = psA (ScalarE copy fp32->fp16), m = ut*sv (DVE fp16 2x),
    m *= psB (DVE, one-PSUM-operand); the d-reduction alternates between
    DVE reduce_sum and ScalarE activation-accumulate to balance engines.
- u-gather pad slots use a duplicated index 0 (NOT -1: num_idxs_reg must
  equal the count of non-negative indices and be core-invariant; fp16
  plain gathers are concurrency-safe across queues, transposed ones are
  NOT and were abandoned).

Numerics: fp16 values, fp32 PSUM/reduction. rel err ~4.6e-4.
Measured: ~293us/pass vs ~596-635us baseline (~2x).
"""

import contextlib
import os
import sys

import numpy as np
import ml_dtypes

sys.path.insert(0, "/opt/trn_rl_repo")

import concourse.bass as bass
import concourse.mybir as mybir
from concourse import bacc
from concourse.tile import TileContext

N_NODES = 10000
N_EDGES = 640000
D = 128
NUM_RELS = 500
N_CORES = 8
N_B = 79
N_RC = 4

F32 = mybir.dt.float32
F16 = mybir.dt.float16
F8 = mybir.dt.float8e4
I16 = mybir.dt.int16
FP8 = np.dtype(ml_dtypes.float8_e4m3)

TCOL = 4096
GP_T = TCOL // 128  # 32 groups per tile


def _wrap(ix_tile: np.ndarray) -> np.ndarray:
    t = ix_tile.shape[0]
    a = ix_tile.astype(np.int16).reshape(t // 16, 16).T
    return np.broadcast_to(a[None], (8, 16, t // 16)).reshape(128, t // 16)


def marshal(src, dst, etype):
    """Global sort/deal/bin-pack. Returns (meta, extras, perm).

    meta = (ntiles, xslot, vxslot, groups) with
    groups[gid] = (tile, gcol, segs, chunks):
      segs   = [(bucket, vxslot_or_-1), ...]  (psA matmuls; buckets are
               bin-packed so a 128-slot group may straddle a boundary)
      chunks = [(rc, xslot_or_-1), ...]       (psB matmuls)
    perm[c, t*TCOL + col] = global edge id (-1 pad).
    """
    b = (dst >> 7).astype(np.int64)
    order = np.lexsort((etype, b))
    bs = b[order]
    gcounts = np.bincount(bs, minlength=N_B)
    starts = np.concatenate([[0], np.cumsum(gcounts)])
    percore = -(-gcounts // N_CORES)
    # bin-packed per-core slot layout: bucket b at [sb[b], sb[b]+percore[b]),
    # no 128-alignment (straddle groups get extra accumulating psA matmuls)
    sb = np.concatenate([[0], np.cumsum(percore)])
    l_used = int(sb[-1])
    ngroups = -(-l_used // 128)
    ntiles = -(-ngroups // GP_T)
    L = ntiles * TCOL

    perm = np.full((N_CORES, L), -1, np.int64)
    bucket_of_slot = np.full(L, -1, np.int64)
    for bb in range(N_B):
        g = gcounts[bb]
        if g == 0:
            continue
        s = starts[bb]
        base = int(sb[bb])
        bucket_of_slot[base : base + percore[bb]] = bb
        p = np.arange(percore[bb])
        for c in range(N_CORES):
            k = p * N_CORES + c
            ok = k < g
            perm[c, base + p[ok]] = order[s + k[ok]]

    # groups: bucket segments + rel-chunk unions (core-invariant)
    rc_all = (etype >> 7).astype(np.int64)
    groups = []
    for gid in range(ngroups):
        t, gcol = gid // GP_T, gid % GP_T
        lanes = gid * 128 + np.arange(128)
        bos = bucket_of_slot[lanes]
        segs = []
        for bb in np.unique(bos):
            if bb >= 0:
                segs.append(int(bb))
        if not segs:
            segs = [0]  # all-pad group: dummy segment
        rcs = set()
        for c in range(N_CORES):
            pp = perm[c, lanes]
            v = pp >= 0
            if v.any():
                rcs.update(np.unique(rc_all[pp[v]]).tolist())
        chunks = sorted(rcs) if rcs else [0]
        groups.append((t, gcol, segs, chunks))

    # x-slots per tile: psB non-primary chunks (owx) and psA non-primary
    # bucket segments (ovx) get side one-hot blocks
    xslot_of, vxslot_of = {}, {}
    xcount = [0] * ntiles
    vxcount = [0] * ntiles
    for gid, (t, gcol, segs, chunks) in enumerate(groups):
        for rc in chunks[1:]:
            xslot_of[(gid, rc)] = xcount[t]
            xcount[t] += 1
        for bb in segs[1:]:
            vxslot_of[(gid, bb)] = vxcount[t]
            vxcount[t] += 1
    xslot = max(xcount) if xcount else 0
    vxslot = max(vxcount) if vxcount else 0

    gmeta = []
    for gid, (t, gcol, segs, chunks) in enumerate(groups):
        sg = [(segs[0], -1)] + [(bb, vxslot_of[(gid, bb)]) for bb in segs[1:]]
        ch = [(chunks[0], -1)] + [(rc, xslot_of[(gid, rc)]) for rc in chunks[1:]]
        gmeta.append((t, gcol, sg, ch))

    # per-core tensors
    extras = []
    prim_rc = np.array([g[3][0][0] for g in gmeta], np.int64)
    prim_bb = np.array([g[2][0][0] for g in gmeta], np.int64)
    for c in range(N_CORES):
        p = perm[c]
        valid = p >= 0
        pc = np.where(valid, p, 0)
        # pad slots use index 0 (not -1): num_idxs_reg must equal the count
        # of non-negative indices, which must be core-invariant
        s_idx = np.where(valid, src[pc], 0).astype(np.int64)
        uix = np.stack(
            [_wrap(s_idx[t * TCOL : (t + 1) * TCOL]) for t in range(ntiles)]
        )
        slots = np.arange(L)
        tsel = slots // TCOL
        csel = slots % TCOL
        # tail slots past the last real group are pads (valid=False); clip
        # so the prim_* lookups stay in bounds
        gsel = np.minimum(slots // 128, len(gmeta) - 1)
        dj = (dst[pc] & 127).astype(np.int64)
        rl = (etype[pc] & 127).astype(np.int64)
        rcv = rc_all[pc]
        bv = bucket_of_slot[slots]

        is_pb = valid & (bv == prim_bb[gsel])
        ov = np.zeros((ntiles, 128, TCOL), FP8)
        ov[tsel[is_pb], dj[is_pb], csel[is_pb]] = 1.0

        ovx = np.zeros((ntiles, 128, max(vxslot, 1) * 128), FP8)
        is_vx = valid & ~is_pb
        if is_vx.any():
            vxs = np.array(
                [vxslot_of[(gsel[s], bv[s])] for s in slots[is_vx]],
                np.int64,
            )
            ovx[tsel[is_vx], dj[is_vx], vxs * 128 + (csel[is_vx] % 128)] = 1.0

        is_prim = valid & (rcv == prim_rc[gsel])
        owm = np.zeros((ntiles, 128, TCOL), FP8)
        owm[tsel[is_prim], rl[is_prim], csel[is_prim]] = 1.0

        owx = np.zeros((ntiles, 128, max(xslot, 1) * 128), FP8)
        is_x = valid & ~is_prim
        if is_x.any():
            xs = np.array(
                [xslot_of[(gsel[s], rcv[s])] for s in slots[is_x]],
                np.int64,
            )
            owx[tsel[is_x], rl[is_x], xs * 128 + (csel[is_x] % 128)] = 1.0
        extras.append(
            {"uix": uix, "ov": ov, "ovx": ovx, "owm": owm, "owx": owx})

    return (ntiles, xslot, vxslot, gmeta), extras, perm


def marshal_tables(h, fwd_rel):
    h16 = h.astype(np.float16)
    hp = np.zeros((N_B * 128, D), np.float16)
    hp[:N_NODES] = h16
    hsb = hp.reshape(N_B, 128, D).transpose(1, 0, 2).copy()  # [j, b, d]
    wpad = np.zeros((N_RC * 128, D), np.float16)
    wpad[:NUM_RELS] = fwd_rel.astype(np.float16)
    wsb = wpad.reshape(N_RC, 128, D).transpose(1, 0, 2).copy()  # [r, rc, d]
    return h16, hsb, wsb


def build_s3(meta, bufs=4, hw_repeat=0):
    ntiles, xslot, vxslot, gmeta = meta
    xcols = max(xslot, 1) * 128
    vxcols = max(vxslot, 1) * 128
    nc = bacc.Bacc(num_swdge_queues=4, dynamic_dma_scratch_size=16384)
    h16 = nc.declare_dram_parameter("h16", [N_NODES, D], F16, isOutput=False)
    hsb = nc.declare_dram_parameter("hsb", [128, N_B, D], F16, isOutput=False)
    wsb = nc.declare_dram_parameter("wsb", [128, N_RC, D], F16, isOutput=False)
    uix = nc.declare_dram_parameter(
        "uix", [ntiles, 128, TCOL // 16], I16, isOutput=False)
    ov = nc.declare_dram_parameter("ov", [ntiles, 128, TCOL], F8, isOutput=False)
    ovx = nc.declare_dram_parameter("ovx", [ntiles, 128, vxcols], F8, isOutput=False)
    owm = nc.declare_dram_parameter("owm", [ntiles, 128, TCOL], F8, isOutput=False)
    owx = nc.declare_dram_parameter("owx", [ntiles, 128, xcols], F8, isOutput=False)
    out = nc.declare_dram_parameter("scores", [ntiles, 128, GP_T], F32, isOutput=True)

    by_tile = {}
    for (t, gcol, sg, ch) in gmeta:
        by_tile.setdefault(t, {})[gcol] = (sg, ch)

    with TileContext(nc) as tc:
        with (
            tc.tile_pool(name="wts", bufs=1) as wp,
            tc.tile_pool(name="io", bufs=bufs) as iop,
            tc.tile_pool(name="sv", bufs=6) as svp,
            tc.tile_pool(name="m", bufs=6) as mp,
            tc.tile_pool(name="stg", bufs=3) as sp,
            tc.tile_pool(name="ps", bufs=4, space=bass.MemorySpace.PSUM) as pab,
        ):
            hsbt = wp.tile([128, N_B, D], F16, tag="hsb")
            wsbt = wp.tile([128, N_RC, D], F16, tag="wsb")
            nc.sync.dma_start(out=hsbt[:], in_=hsb[:])
            nc.sync.dma_start(out=wsbt[:], in_=wsb[:])

            loop_ctx = (
                tc.For_i(0, hw_repeat) if hw_repeat else contextlib.nullcontext()
            )
            with loop_ctx:
                q = 0
                for t in range(ntiles):
                    # ix stays in the shared iop pool: the per-tile ix DMA
                    # paces SWDGE desc-gen (more in-flight descriptors
                    # measurably SLOWS the Q7 desc-gen loop)
                    ix = iop.tile([128, TCOL // 16], I16, tag="ix")
                    nc.sync.dma_start(out=ix[:], in_=uix[t])
                    ut = iop.tile([128, GP_T, D], F16, tag="ut")
                    for k in range(2):
                        nc.gpsimd.dma_gather(
                            out_ap=ut[:, k * 16 : (k + 1) * 16, :],
                            in_ap=h16[:],
                            idxs_ap=ix[:, k * 128 : (k + 1) * 128],
                            num_idxs=2048, num_idxs_reg=2048, elem_size=D,
                            single_packet=False, queue_num=q % 4,
                        )
                        q += 1
                    ovt = iop.tile([128, TCOL], F8, tag="ovt")
                    ovxt = iop.tile([128, vxcols], F8, tag="ovxt")
                    owmt = iop.tile([128, TCOL], F8, tag="owmt")
                    owxt = iop.tile([128, xcols], F8, tag="owxt")
                    nc.sync.dma_start(out=ovt[:], in_=ov[t])
                    nc.sync.dma_start(out=ovxt[:], in_=ovx[t])
                    nc.sync.dma_start(out=owmt[:], in_=owm[t])
                    nc.sync.dma_start(out=owxt[:], in_=owx[t])

                    stag = sp.tile([128, GP_T], F32, tag="stg")
                    tgroups = by_tile.get(t, {})
                    n_g = len(tgroups)
                    # present groups are a contiguous prefix 0..n_g-1
                    assert sorted(tgroups) == list(range(n_g)), (t, sorted(tgroups))
                    for mq in range(GP_T // 4):
                        npres = min(4, n_g - mq * 4)
                        if npres <= 0:
                            break
                        psA = pab.tile([128, 4, D], F32, tag="psA")
                        psB = pab.tile([128, 4, D], F32, tag="psB")
                        for i in range(npres):
                            g = mq * 4 + i
                            sg, ch = tgroups[g]
                            gsl = slice(g * 128, (g + 1) * 128)
                            for si, (bb, vxs) in enumerate(sg):
                                lhsa = (
                                    ovt[:, gsl] if vxs < 0
                                    else ovxt[:, vxs * 128 : (vxs + 1) * 128]
                                )
                                nc.tensor.matmul(
                                    psA[:, i, :], lhsa, hsbt[:, bb, :],
                                    start=(si == 0), stop=(si == len(sg) - 1),
                                )
                            for ci, (rc, xs) in enumerate(ch):
                                lhs = (
                                    owmt[:, gsl] if xs < 0
                                    else owxt[:, xs * 128 : (xs + 1) * 128]
                                )
                                nc.tensor.matmul(
                                    psB[:, i, :], lhs, wsbt[:, rc, :],
                                    start=(ci == 0), stop=(ci == len(ch) - 1),
                                )
                        sv = svp.tile([128, 4, D], F16, tag="sv")
                        nc.scalar.activation(
                            out=sv[:, :npres, :], in_=psA[:, :npres, :],
                            func=mybir.ActivationFunctionType.Copy,
                        )
                        mm = mp.tile([128, 4, D], F16, tag="mm")
                        nc.vector.tensor_mul(
                            mm[:, :npres, :],
                            ut[:, mq * 4 : mq * 4 + npres, :],
                            sv[:, :npres, :])
                        nc.vector.tensor_mul(
                            mm[:, :npres, :], mm[:, :npres, :],
                            psB[:, :npres, :])
                        if mq % 2 == 0:
                            nc.vector.reduce_sum(
                                stag[:, mq * 4 : mq * 4 + npres],
                                mm[:, :npres, :],
                                axis=mybir.AxisListType.X,
                            )
                        else:
                            # balance: odd macros reduce on ScalarE (per
                            # group, activation free-dim accumulate)
                            dump = svp.tile([128, 4, D], F16, tag="dump")
                            for i in range(npres):
                                nc.scalar.activation(
                                    out=dump[:, i, :], in_=mm[:, i, :],
                                    func=mybir.ActivationFunctionType.Copy,
                                    accum_out=stag[
                                        :, mq * 4 + i : mq * 4 + i + 1],
                                )
                    if n_g < GP_T:
                        # fill dropped tail cols so the DMA-out reads
                        # initialized SBUF
                        nc.gpsimd.memset(stag[:, n_g:], 0.0)
                    nc.sync.dma_start(out=out[t], in_=stag[:])

    nc.compile()
    return nc


_CACHE = {}
LAST_RESULTS = None


def kernel(h, src, dst, etype, fwd_rel, rev_rel=None):
    global LAST_RESULTS
    from concourse.bass_utils import run_bass_kernel_spmd

    h = np.asarray(h, dtype=np.float32)
    fwd_rel = np.asarray(fwd_rel, dtype=np.float32)
    src = np.asarray(src).astype(np.int64)
    dst = np.asarray(dst).astype(np.int64)
    etype = np.asarray(etype).astype(np.int64)

    meta, extras, perm = marshal(src, dst, etype)
    key = (meta[0], meta[1], meta[2], tuple(
        (t, g, tuple(map(tuple, sg)), tuple(map(tuple, ch)))
        for (t, g, sg, ch) in meta[3]))
    if key not in _CACHE:
        _CACHE[key] = build_s3(meta)
    nc = _CACHE[key]

    h16, hsb, wsb = marshal_tables(h, fwd_rel)
    in_maps = [
        {"h16": h16, "hsb": hsb, "wsb": wsb, **extras[c]} for c in range(N_CORES)
    ]
    res = run_bass_kernel_spmd(
        nc, in_maps, core_ids=list(range(N_CORES)),
        trace=bool(os.environ.get("KERNEL_TRACE")),
    )
    LAST_RESULTS = res

    ntiles = meta[0]
    scores = np.empty(N_EDGES, np.float32)
    for c in range(N_CORES):
        got = res.results[c]["scores"]  # [nt, 128(lane), 32(g)]
        flat = got.transpose(0, 2, 1).reshape(-1)  # slot = t*4096 + g*128 + lane
        p = perm[c]
        v = p >= 0
        scores[p[v]] = flat[v]
    return scores

